# revision 5
# baseline (speedup 1.0000x reference)
"""Trainium2 Bass kernel for causal multi-head self-attention + output proj.

Problem: x [4, 2048, 2048], w_q/w_k/w_v/w_o [2048, 2048], NH=16 heads, HD=128,
causal softmax(QK^T/sqrt(128)) V, then o @ w_o.T.

Sharding over 8 NeuronCores: core c handles batch c//2 and heads
(c%2)*8 .. +8 (tensor parallel over heads). Each core computes its partial
output projection; pairs of cores all-reduce on-chip so every core holds the
full [2048, 2048] output for its batch.

Per-core kernel (all matmuls in float32r = FP22, full PE rate):
  Phase A (per group of 2 heads): stream x^T in [2048c, 512s] panels, compute
    QT/KT [d, s] per head and V [k, d] via PE; then attention per head:
    scores^T[k, q] = KT_blk.T @ QT_blk (no transposes anywhere), exp on ACT,
    causal mask via a triangular tile on DVE, softmax denominators via
    ones-vector matmuls accumulated on the PE, attention output o^T[d, q]
    accumulated on the PE, normalization via PE row-broadcast + DVE multiply.
  Phase B: out[q, j] = sum_h oT_h.T @ w_oT_h, streamed from a DRAM spill.
  Collective: pairwise AllReduce(add) of the partial outputs.
"""

import sys

for _p in ("/opt/trn_rl_repo",):
    if _p not in sys.path:
        sys.path.insert(0, _p)

import numpy as np

import concourse.bass as bass
import concourse.tile as tile
from concourse import bacc, mybir
from concourse.bass_utils import run_bass_kernel_spmd

F32R = mybir.dt.float32r
F32 = mybir.dt.float32

B, S, H, NH = 4, 2048, 2048, 16
HD = H // NH  # 128
N_CORES = 8
HLOC = NH // 2  # heads per core: 8
CLOC = HLOC * HD  # local channels: 1024
QB = 512  # q block (matmul moving dim)
NQB = S // QB  # 4
NCT = H // 128  # 16 c-tiles (contraction)
NKB = S // 128  # 16 k tiles
GROUPS = HLOC // 2  # 4 groups of 2 heads

SCALE = float(np.float32(1.0) / np.sqrt(np.float32(HD)))

_NC_CACHE = None


def _build():
    nc = bacc.Bacc("TRN2", target_bir_lowering=False, debug=False, num_devices=N_CORES)

    xT = nc.dram_tensor("xT", [H, S], F32R, kind="ExternalInput").ap()
    wq = nc.dram_tensor("wq", [H, CLOC], F32R, kind="ExternalInput").ap()
    wk = nc.dram_tensor("wk", [H, CLOC], F32R, kind="ExternalInput").ap()
    wv = nc.dram_tensor("wv", [H, CLOC], F32R, kind="ExternalInput").ap()
    wo = nc.dram_tensor("wo", [CLOC, H], F32R, kind="ExternalInput").ap()
    ones = nc.dram_tensor("ones", [128, 128], F32R, kind="ExternalInput").ap()
    out = nc.dram_tensor("out", [S, H], F32, kind="ExternalOutput").ap()

    oT_spill = nc.dram_tensor("oT_spill", [CLOC, S], F32R).ap()
    out_bounce = nc.dram_tensor("out_bounce", [S, H], F32).ap()
    out_shared = nc.dram_tensor("out_shared", [S, H], F32).ap()

    with tile.TileContext(nc) as tc:
        with (
            tc.tile_pool(name="const", bufs=1) as const_pool,
            tc.tile_pool(name="xpanel", bufs=24) as xpanel_pool,
            tc.tile_pool(name="w", bufs=NCT) as w_pool,
            tc.tile_pool(name="qk", bufs=2) as qk_pool,
            tc.tile_pool(name="v", bufs=NKB) as v_pool,
            tc.tile_pool(name="exp", bufs=3) as exp_pool,
            tc.tile_pool(name="small", bufs=2) as small_pool,
            tc.tile_pool(name="ps_proj", bufs=2, space="PSUM") as ps_proj,
            tc.tile_pool(name="ps_s", bufs=2, space="PSUM") as ps_s,
            tc.tile_pool(name="ps_o", bufs=2, space="PSUM") as ps_o,
            tc.tile_pool(name="ps_l", bufs=1, space="PSUM") as ps_l,
            tc.tile_pool(name="ps_b", bufs=1, space="PSUM") as ps_b,
        ):
            ones_t = const_pool.tile([128, 128], F32R)
            nc.sync.dma_start(ones_t[:], ones[:])
            # causal masks for the 4 possible diagonal positions within a
            # [k=128, q=512] tile: ones where q >= k, i.e. f - 128*j0 - p >= 0
            masks = []
            for j0 in range(4):
                m = const_pool.tile([128, QB], F32, name=f"mask{j0}")
                nc.gpsimd.memset(m[:], 1.0)
                nc.gpsimd.affine_select(
                    out=m[:],
                    in_=m[:],
                    compare_op=mybir.AluOpType.is_ge,
                    fill=0.0,
                    base=-128 * j0,
                    channel_multiplier=-1,
                    pattern=[[1, QB]],
                )
                masks.append(m)

            for g in range(GROUPS):
                # --- group weights: [128, 256] c-tiles, cols = 2 heads ---
                wq_t, wk_t, wv_t = [], [], []
                for ci in range(NCT):
                    cs = slice(ci * 128, (ci + 1) * 128)
                    gs = slice(g * 256, (g + 1) * 256)
                    t = w_pool.tile([128, 256], F32R, tag="wq")
                    nc.sync.dma_start(t[:], wq[cs, gs])
                    wq_t.append(t)
                    t = w_pool.tile([128, 256], F32R, tag="wk")
                    nc.sync.dma_start(t[:], wk[cs, gs])
                    wk_t.append(t)
                    t = w_pool.tile([128, 256], F32R, tag="wv")
                    nc.sync.dma_start(t[:], wv[cs, gs])
                    wv_t.append(t)

                qt_t = [qk_pool.tile([128, S], F32R, tag="qt", name=f"qt{g}_{i}") for i in range(2)]
                kt_t = [qk_pool.tile([128, S], F32R, tag="kt", name=f"kt{g}_{i}") for i in range(2)]
                v_t = [v_pool.tile([128, 256], F32R, tag="v", name=f"v{g}_{i}") for i in range(NKB)]

                # --- projections, streaming x^T in [2048, 512] panels ---
                for p in range(NQB):
                    xp = []
                    for ci in range(NCT):
                        t = xpanel_pool.tile([128, QB], F32R, tag="xp")
                        nc.sync.dma_start(
                            t[:], xT[ci * 128 : (ci + 1) * 128, p * QB : (p + 1) * QB]
                        )
                        xp.append(t)
                    for hl in range(2):
                        hs = slice(hl * 128, (hl + 1) * 128)
                        ps = ps_proj.tile([128, QB], F32, tag="ps")
                        for ci in range(NCT):
                            nc.tensor.matmul(
                                ps[:],
                                wq_t[ci][:, hs],
                                xp[ci][:],
                                start=(ci == 0),
                                stop=(ci == NCT - 1),
                            )
                        nc.scalar.copy(qt_t[hl][:, p * QB : (p + 1) * QB], ps[:])
                        ps = ps_proj.tile([128, QB], F32, tag="ps")
                        for ci in range(NCT):
                            nc.tensor.matmul(
                                ps[:],
                                wk_t[ci][:, hs],
                                xp[ci][:],
                                start=(ci == 0),
                                stop=(ci == NCT - 1),
                            )
                        nc.scalar.copy(kt_t[hl][:, p * QB : (p + 1) * QB], ps[:])
                    for kk in range(4):
                        kb = p * 4 + kk
                        ps = ps_proj.tile([128, 256], F32, tag="ps")
                        for ci in range(NCT):
                            nc.tensor.matmul(
                                ps[:],
                                xp[ci][:, kk * 128 : (kk + 1) * 128],
                                wv_t[ci][:],
                                start=(ci == 0),
                                stop=(ci == NCT - 1),
                            )
                        nc.scalar.copy(v_t[kb][:], ps[:])

                # --- attention per head ---
                for hl in range(2):
                    h = 2 * g + hl
                    hs = slice(hl * 128, (hl + 1) * 128)
                    for qb in range(NQB):
                        nki = 4 * qb + 4
                        l_ps = ps_l.tile([1, QB], F32, tag="l")
                        o_ps = ps_o.tile([128, QB], F32, tag="o")
                        for ki in range(nki):
                            s_ps = ps_s.tile([128, QB], F32, tag="s")
                            nc.tensor.matmul(
                                s_ps[:],
                                kt_t[hl][:, ki * 128 : (ki + 1) * 128],
                                qt_t[hl][:, qb * QB : (qb + 1) * QB],
                                start=True,
                                stop=True,
                            )
                            e_t = exp_pool.tile([128, QB], F32R, tag="e")
                            nc.scalar.activation(
                                e_t[:],
                                s_ps[:],
                                mybir.ActivationFunctionType.Exp,
                                scale=SCALE,
                            )
                            if ki >= 4 * qb:
                                j0 = ki - 4 * qb
                                nc.vector.tensor_mul(e_t[:], e_t[:], masks[j0][:])
                            nc.tensor.matmul(
                                l_ps[:],
                                ones_t[:, 0:1],
                                e_t[:],
                                start=(ki == 0),
                                stop=(ki == nki - 1),
                                skip_group_check=True,
                            )
                            nc.tensor.matmul(
                                o_ps[:],
                                v_t[ki][:, hs],
                                e_t[:],
                                start=(ki == 0),
                                stop=(ki == nki - 1),
                                skip_group_check=True,
                            )
                        l_sb = small_pool.tile([1, QB], F32R, tag="l_sb")
                        nc.scalar.copy(l_sb[:], l_ps[:])
                        b_ps = ps_b.tile([128, QB], F32, tag="b")
                        nc.tensor.matmul(
                            b_ps[:], ones_t[0:1, :], l_sb[:], start=True, stop=True
                        )
                        r_sb = small_pool.tile([128, QB], F32, tag="r_sb")
                        nc.vector.reciprocal(r_sb[:], b_ps[:])
                        ot = small_pool.tile([128, QB], F32R, tag="ot")
                        nc.vector.tensor_mul(ot[:], o_ps[:], r_sb[:])
                        nc.sync.dma_start(
                            oT_spill[
                                h * 128 : (h + 1) * 128, qb * QB : (qb + 1) * QB
                            ],
                            ot[:],
                        )

        # --- phase B: out[q, j] = sum_h oT_h.T @ w_oT_h ---
        with (
            tc.tile_pool(name="wo", bufs=HLOC) as wo_pool,
            tc.tile_pool(name="oq", bufs=2 * HLOC) as oq_pool,
            tc.tile_pool(name="st", bufs=3) as st_pool,
            tc.tile_pool(name="ps_out", bufs=4, space="PSUM") as ps_out,
        ):
            wo_t = []
            for hh in range(HLOC):
                t = wo_pool.tile([128, H], F32R, tag="wo")
                nc.sync.dma_start(t[:], wo[hh * 128 : (hh + 1) * 128, :])
                wo_t.append(t)
            for qb in range(NQB):
                oq = []
                for hh in range(HLOC):
                    t = oq_pool.tile([128, QB], F32R, tag="oq")
                    nc.sync.dma_start(
                        t[:],
                        oT_spill[hh * 128 : (hh + 1) * 128, qb * QB : (qb + 1) * QB],
                    )
                    oq.append(t)
                for qi in range(4):
                    q0 = qb * QB + qi * 128
                    for j in range(NQB):
                        ps = ps_out.tile([128, QB], F32, tag="po")
                        for hh in range(HLOC):
                            nc.tensor.matmul(
                                ps[:],
                                oq[hh][:, qi * 128 : (qi + 1) * 128],
                                wo_t[hh][:, j * QB : (j + 1) * QB],
                                start=(hh == 0),
                                stop=(hh == HLOC - 1),
                            )
                        st = st_pool.tile([128, QB], F32, tag="st")
                        nc.scalar.copy(st[:], ps[:])
                        nc.sync.dma_start(
                            out_bounce[q0 : q0 + 128, j * QB : (j + 1) * QB], st[:]
                        )

    # --- pairwise all-reduce of partial outputs, then final output ---
    with (
        nc.Block() as block,
        nc.semaphore("cc_sem") as cc_sem,
        nc.semaphore("dma_sem") as dma_sem,
    ):

        @block.gpsimd
        def _(gpsimd):
            gpsimd.collective_compute(
                "AllReduce",
                mybir.AluOpType.add,
                replica_groups=[[0, 1], [2, 3], [4, 5], [6, 7]],
                ins=[out_bounce[:]],
                outs=[out_shared[:]],
            ).then_inc(cc_sem, 1)
            gpsimd.wait_ge(cc_sem, 1)
            gpsimd.dma_start(out=out[:], in_=out_shared[:]).then_inc(dma_sem, 16)
            gpsimd.wait_ge(dma_sem, 16)

    nc.compile()
    return nc


def kernel(x, w_q, w_k, w_v, w_o):
    global _NC_CACHE
    if _NC_CACHE is None:
        _NC_CACHE = _build()
    nc = _NC_CACHE

    x = np.asarray(x, dtype=np.float32)
    w_q = np.asarray(w_q, dtype=np.float32)
    w_k = np.asarray(w_k, dtype=np.float32)
    w_v = np.asarray(w_v, dtype=np.float32)
    w_o = np.asarray(w_o, dtype=np.float32)

    ones = np.ones((128, 128), dtype=np.float32)
    xTs = [np.ascontiguousarray(x[b].T) for b in range(B)]
    wqs = [np.ascontiguousarray(w_q[i * CLOC : (i + 1) * CLOC, :].T) for i in range(2)]
    wks = [np.ascontiguousarray(w_k[i * CLOC : (i + 1) * CLOC, :].T) for i in range(2)]
    wvs = [np.ascontiguousarray(w_v[i * CLOC : (i + 1) * CLOC, :].T) for i in range(2)]
    wos = [np.ascontiguousarray(w_o[:, i * CLOC : (i + 1) * CLOC].T) for i in range(2)]

    in_maps = []
    for c in range(N_CORES):
        b, hh = c // 2, c % 2
        in_maps.append(
            {
                "xT": xTs[b],
                "wq": wqs[hh],
                "wk": wks[hh],
                "wv": wvs[hh],
                "wo": wos[hh],
                "ones": ones,
            }
        )

    res = run_bass_kernel_spmd(nc, in_maps, list(range(N_CORES)))
    out = np.empty((B, S, H), dtype=np.float32)
    for b in range(B):
        out[b] = res.results[2 * b]["out"]
    return out
